# revision 16
# baseline (speedup 1.0000x reference)
"""Trainium2 Bass kernel for nn_BSquareModel (45 pairwise binary MLP classifiers + voting).

Math: for each of E=45 class pairs (c1,c2):
  h1 = relu(x @ W1[e] + b1[e]);  h2 = relu(h1 @ W2[e] + b2[e])
  diff = h2 @ (Wout[e,:,0]-Wout[e,:,1]) + (bout[e,0]-bout[e,1])
  vote goes to c1 if diff >= 0 else c2; output = per-class vote counts [B, 10].

Sharding: data-parallel over batch B=8192 across 8 cores (1024 rows each),
weights replicated. Device computes in bf16 (matmul full rate) with fp32 PSUM
accumulation, keeping activations in [feature, batch] layout so the contraction
dim always sits on SBUF partitions. The vote scatter is a tiny matmul against a
{-1,0,+1} incidence matrix (plus a constant-offset row). Because the output is
integer votes, only samples with |diff| below a threshold can be affected by
bf16 rounding; those few are recomputed exactly in fp32 on the host and the
votes corrected.
"""

import numpy as np
import ml_dtypes

import concourse.bass as bass
import concourse.tile as tile
from concourse import bacc, mybir
from concourse.bass_utils import run_bass_kernel_spmd

NUM_CLASSES = 10
B = 8192
IN = 784
HID = 128
E = 45
N_CORES = 8
BS = B // N_CORES          # 1024 batch rows per core
CHUNK = 512                # matmul moving-dim chunk (one PSUM bank)
NCHUNK = BS // CHUNK       # 2
KT = 7                     # ceil(784/128) contraction tiles for layer 1
KPAD = KT * 128            # 896
# |diff| threshold below which the bf16 device result could mis-vote; those
# samples are recomputed in fp32 on the host. Calibrated against measured
# max |device_diff - fp32_diff| (see test.py); keep a ~4x safety margin.
TAU = 0.15

BF16 = ml_dtypes.bfloat16
_C1, _C2 = np.triu_indices(NUM_CLASSES, k=1)

_CACHE = {}


def build_nc():
    if "nc" in _CACHE:
        return _CACHE["nc"]
    f32 = mybir.dt.float32
    bf16 = mybir.dt.bfloat16

    nc = bacc.Bacc("TRN2", target_bir_lowering=False, debug=False, num_devices=N_CORES)

    xT = nc.declare_dram_parameter("xT", [KT, 128, BS], bf16, isOutput=False)
    W1p = nc.declare_dram_parameter("W1p", [128, E * KT * HID], bf16, isOutput=False)
    W2p = nc.declare_dram_parameter("W2p", [128, E * HID], bf16, isOutput=False)
    # masked diff weights: wdM[p, e, j] = wd[e, p] if j == e else 0 — so the 45
    # diff matmuls (M=45 each) accumulate into one [45, CHUNK] PSUM tile with
    # each classifier landing on its own row (PE can't write at partition e).
    wdM = nc.declare_dram_parameter("wdM", [128, E * E], bf16, isOutput=False)
    b1T = nc.declare_dram_parameter("b1T", [128, E], f32, isOutput=False)
    b2T = nc.declare_dram_parameter("b2T", [128, E], f32, isOutput=False)
    bdv = nc.declare_dram_parameter("bdv", [E, 1], f32, isOutput=False)
    Mm = nc.declare_dram_parameter("Mm", [E + 1, NUM_CLASSES], bf16, isOutput=False)
    votes = nc.declare_dram_parameter("votes", [BS, NUM_CLASSES], f32, isOutput=True)
    dqv = nc.declare_dram_parameter("dqv", [E, BS], f32, isOutput=True)

    with tile.TileContext(nc) as tc:
        with (
            tc.tile_pool(name="consts", bufs=1) as consts,
            tc.tile_pool(name="acts", bufs=3) as acts,
            tc.tile_pool(name="small", bufs=2) as small,
            tc.tile_pool(name="pz1", bufs=2, space="PSUM") as pz1p,
            tc.tile_pool(name="pz2", bufs=2, space="PSUM") as pz2p,
            tc.tile_pool(name="pdiff", bufs=1, space="PSUM") as pdiffp,
            tc.tile_pool(name="pvotes", bufs=2, space="PSUM") as pvp,
        ):
            # DMAs are spread across engine queues (each issue costs ~600ns of
            # queue time) and ordered so the first classifiers' data lands first:
            # xts on scalar + W1[e0] on sync arrive in parallel -> PE starts ~9us.
            xts = consts.tile([128, KT, BS], bf16)
            for k in range(KT):
                nc.scalar.dma_start(out=xts[:, k, :], in_=xT[k])

            # W1 on the sync HWDGE queue: first few classifiers individually so
            # compute starts early, rest batched into fewer, larger transfers.
            w1s = consts.tile([128, E, KT, HID], bf16)
            w1v = W1p[:].rearrange("p (e k h) -> p e k h", e=E, k=KT)
            w1_groups = [(0, 1), (1, 2), (2, 3), (3, 5)]
            w1_groups += [(s, min(s + 8, E)) for s in range(5, E, 8)]
            for s, t in w1_groups:
                nc.sync.dma_start(out=w1s[:, s:t, :, :], in_=w1v[:, s:t, :, :])

            b1s = consts.tile([128, E], f32)
            nc.scalar.dma_start(out=b1s, in_=b1T[:])
            b2s = consts.tile([128, E], f32)
            nc.scalar.dma_start(out=b2s, in_=b2T[:])
            bds = consts.tile([E, 1], f32)
            nc.scalar.dma_start(out=bds, in_=bdv[:])
            mms = consts.tile([E + 1, NUM_CLASSES], bf16)
            nc.scalar.dma_start(out=mms, in_=Mm[:])

            # w2/wd split so the first classifiers' layer-2 + diff weights land
            # before they're needed (~13us in), remainder streams behind.
            w2s = consts.tile([128, E, HID], bf16)
            w2v = W2p[:].rearrange("p (e h) -> p e h", e=E)
            wds = consts.tile([128, E, E], bf16)
            wdv = wdM[:].rearrange("p (e j) -> p e j", e=E)
            for s, t in [(0, 8), (8, 24), (24, E)]:
                nc.scalar.dma_start(out=w2s[:, s:t, :], in_=w2v[:, s:t, :])
                nc.scalar.dma_start(out=wds[:, s:t, :], in_=wdv[:, s:t, :])

            # e-outer / chunk-inner: each classifier's weights feed both batch
            # chunks back-to-back, halving the required W1 delivery rate.
            pdiffs = [
                pdiffp.tile([E, CHUNK], mybir.dt.float32, name=f"pdiff{c}", tag=f"pdiff{c}")
                for c in range(NCHUNK)
            ]
            for e in range(E):
                for c in range(NCHUNK):
                    cs = bass.ts(c, CHUNK)
                    z1 = pz1p.tile([128, CHUNK], mybir.dt.float32)
                    for k in range(KT):
                        nc.tensor.matmul(
                            z1,
                            lhsT=w1s[:, e, k, :],
                            rhs=xts[:, k, cs],
                            start=(k == 0),
                            stop=(k == KT - 1),
                        )
                    h1 = acts.tile([128, CHUNK], bf16, tag="h1")
                    nc.scalar.activation(
                        h1, z1, mybir.ActivationFunctionType.Relu, bias=b1s[:, e : e + 1]
                    )
                    z2 = pz2p.tile([128, CHUNK], mybir.dt.float32)
                    nc.tensor.matmul(z2, lhsT=w2s[:, e, :], rhs=h1, start=True, stop=True)
                    h2 = acts.tile([128, CHUNK], bf16, tag="h2")
                    nc.vector.tensor_scalar(
                        h2, z2, b2s[:, e : e + 1], 0.0,
                        op0=mybir.AluOpType.add, op1=mybir.AluOpType.max,
                    )
                    nc.tensor.matmul(
                        pdiffs[c], lhsT=wds[:, e, :], rhs=h2,
                        start=(e == 0), stop=(e == E - 1),
                    )

            for c in range(NCHUNK):
                cs = bass.ts(c, CHUNK)
                diffb = small.tile([E, CHUNK], mybir.dt.float32, tag="diffb")
                nc.vector.tensor_scalar(
                    diffb, pdiffs[c], bds, None, op0=mybir.AluOpType.add
                )
                nc.gpsimd.dma_start(out=dqv[:, cs], in_=diffb)

                ges = small.tile([E + 1, CHUNK], bf16, tag="ges")
                # row E must be all-ones (constant-offset row of Mm); engines
                # can't address a lone partition 45, so fill then overwrite.
                nc.vector.memset(ges, 1.0)
                nc.vector.tensor_scalar(
                    ges[:E, :], diffb, 0.0, None, op0=mybir.AluOpType.is_ge
                )

                nt = CHUNK // 128
                vsb = small.tile([128, nt, NUM_CLASSES], mybir.dt.float32, tag="vsb")
                for t in range(nt):
                    pv = pvp.tile([128, NUM_CLASSES], mybir.dt.float32)
                    nc.tensor.matmul(
                        pv, lhsT=ges[:, bass.ts(t, 128)], rhs=mms, start=True, stop=True
                    )
                    nc.scalar.copy(vsb[:, t, :], pv)
                nc.sync.dma_start(
                    out=votes[cs, :].rearrange("(t p) o -> p t o", p=128),
                    in_=vsb,
                )
    nc.finalize()
    _CACHE["nc"] = nc
    return nc


def _pack_inputs(x, W1, b1, W2, b2, Wout, bout):
    """Host-side packing into the device layouts (bf16, padded, partition-major)."""
    xTpad = np.zeros((KPAD, B), np.float32)
    xTpad[:IN] = x.T
    xts = xTpad.reshape(KT, 128, B).astype(BF16)

    W1pad = np.zeros((E, KPAD, HID), np.float32)
    W1pad[:, :IN] = W1
    W1p = np.ascontiguousarray(
        W1pad.reshape(E, KT, 128, HID).transpose(2, 0, 1, 3)
    ).astype(BF16).reshape(128, E * KT * HID)

    W2p = np.ascontiguousarray(W2.transpose(1, 0, 2)).astype(BF16).reshape(128, E * HID)

    wd = (Wout[:, :, 0] - Wout[:, :, 1]).astype(np.float32)      # [E, HID]
    bd = (bout[:, 0] - bout[:, 1]).astype(np.float32)            # [E]
    wdM = np.zeros((128, E, E), np.float32)
    wdM[:, np.arange(E), np.arange(E)] = wd.T
    wdM = wdM.astype(BF16).reshape(128, E * E)
    b1T = np.ascontiguousarray(b1.T).astype(np.float32)
    b2T = np.ascontiguousarray(b2.T).astype(np.float32)

    Mm = np.zeros((E + 1, NUM_CLASSES), np.float32)
    Mm[np.arange(E), _C1] += 1.0
    Mm[np.arange(E), _C2] -= 1.0
    Mm[E] = np.arange(NUM_CLASSES)
    Mm = Mm.astype(BF16)

    common = {
        "W1p": W1p, "W2p": W2p, "wdM": wdM,
        "b1T": b1T, "b2T": b2T, "bdv": bd[:, None].copy(), "Mm": Mm,
    }
    in_maps = []
    for c in range(N_CORES):
        m = dict(common)
        m["xT"] = np.ascontiguousarray(xts[:, :, c * BS : (c + 1) * BS])
        in_maps.append(m)
    return in_maps, wd, bd


def run_device(x, W1, b1, W2, b2, Wout, bout, trace=False):
    """Returns (votes [B,10] f32, diff [E,B] f32, BassKernelResults)."""
    in_maps, wd, bd = _pack_inputs(x, W1, b1, W2, b2, Wout, bout)
    nc = build_nc()
    res = run_bass_kernel_spmd(nc, in_maps, list(range(N_CORES)), trace=trace)
    votes = np.concatenate([res.results[c]["votes"] for c in range(N_CORES)], axis=0)
    diff = np.concatenate([res.results[c]["dqv"] for c in range(N_CORES)], axis=1)
    return votes.astype(np.float32), diff, res


def _refine(votes, diff, x, W1, b1, W2, b2, wd, bd):
    """Recompute near-boundary samples in fp32 and patch the vote counts."""
    cand = np.abs(diff) < TAU
    for e in np.nonzero(cand.any(axis=1))[0]:
        idx = np.nonzero(cand[e])[0]
        h = np.maximum(x[idx] @ W1[e] + b1[e], 0.0)
        h = np.maximum(h @ W2[e] + b2[e], 0.0)
        de = h @ wd[e] + bd[e]
        ge_new = de >= 0.0
        ge_old = diff[e, idx] >= 0.0
        flip = ge_new != ge_old
        if flip.any():
            fi = idx[flip]
            sgn = np.where(ge_new[flip], 1.0, -1.0).astype(np.float32)
            np.add.at(votes, (fi, np.full(fi.shape, _C1[e])), sgn)
            np.add.at(votes, (fi, np.full(fi.shape, _C2[e])), -sgn)
    return votes


def kernel(x, W1, b1, W2, b2, Wout, bout):
    x = np.asarray(x, np.float32)
    W1 = np.asarray(W1, np.float32)
    b1 = np.asarray(b1, np.float32)
    W2 = np.asarray(W2, np.float32)
    b2 = np.asarray(b2, np.float32)
    Wout = np.asarray(Wout, np.float32)
    bout = np.asarray(bout, np.float32)

    votes, diff, _ = run_device(x, W1, b1, W2, b2, Wout, bout, trace=False)
    wd = (Wout[:, :, 0] - Wout[:, :, 1]).astype(np.float32)
    bd = (bout[:, 0] - bout[:, 1]).astype(np.float32)
    votes = _refine(votes, diff, x, W1, b1, W2, b2, wd, bd)
    return votes
